# revision 1
# baseline (speedup 1.0000x reference)
"""BinsChamferLoss Trainium2 kernel.

Problem: bins [4,257], target_depth_maps [4,240,320] ->
scalar chamfer loss between per-image bin centers (256 1-D points) and
the valid depth pixels (76800 1-D points per image).

Sharding: the 76800-pixel dim is split across 8 cores (9600 pixels each),
all 4 images and all 256 bins on every core. Host combine is a tiny
min/sum over per-core partials.

Per-core device pipeline:
  cham_y ([part=points, free=bins], partition p owns batch p//32):
    one custom dual-stream DVE op per point column j:
    body=min((bc_lo-t_j)^2, (bc_hi-t_j)^2) with accum_out=min
    -> dy_j in ~(126+128) VEC cycles (2 bins/cycle via both read ports).
  cham_x ([part=bins, free=points]):
    DMA broadcasts each masked t row (via a DRAM bounce) to [128, 9600],
    ACT computes Square(t + (-bc_p)) -> bf16, one custom dual-stream
    min(a,b)+min-accum DVE op reduces each chunk (2 points/cycle).
Invalid points (t < 0.001) are pushed 1e9 away so they never win a min
and their dy contribution is masked out of the sum.
"""

import os
import sys

import numpy as np

sys.path.insert(0, "/opt/trn_rl_repo")

N_CORES = 8
N, P = 4, 256  # batches, bins
L = 240 * 320  # 76800 points per batch
L_LOC = L // N_CORES  # 9600 per core
COLS = (N * L_LOC) // 128  # 300 point-columns per partition
PARTS_PER_BATCH = 128 // N  # 32
FD = 512  # matmul free-dim tile
NTILE = L_LOC // FD  # 18.75 -> handle remainder
_CACHE = {}

_CHAMY_NAME = "CHAMY2_SQDIFF_MINRED_ANT"


def _chamy_ref(in0, in1, c0, c1, c2):
    c0 = np.asarray(c0, np.float32).reshape(-1, 1)
    P_ = in0.shape[0]
    a = (in0.astype(np.float32).reshape(P_, -1) - c0) ** 2
    b = (in1.astype(np.float32).reshape(P_, -1) - c0) ** 2
    body = np.minimum(a, b).astype(np.float32)
    c1 = np.asarray(c1, np.float32).reshape(-1, 1)
    acc = np.minimum(body.min(axis=-1, keepdims=True), c1)
    return body.reshape(in0.shape), acc


def _chamy_op():
    """Register (idempotently) the dual-stream fused
    min((a-s)^2, (b-s)^2) + min-reduce DVE op."""
    from concourse.dve_ops import (CUSTOM_DVE_SPECS, OPS,
                                   _SUB_OPCODE_FOR_NAME, DveOp)
    from concourse.dve_spec import C0, C1, Spec, Src0, Src1, lower, minn, sq
    from concourse.dve_uop import DveOpSpec

    if _CHAMY_NAME in _SUB_OPCODE_FOR_NAME:
        return next(o for o in OPS if o.name == _CHAMY_NAME)
    spec = Spec(body=minn(sq(Src0 - C0), sq(Src1 - C0)), accum=minn,
                accum_init=C1, reference=_chamy_ref)
    row = 1 + len(OPS)
    shas = {}
    for ver in ("v3", "v4"):
        s = DveOpSpec(name=_CHAMY_NAME, opcode=row,
                      uops=lower(spec, ver=ver), rd1_en=True)
        shas[ver] = s.sha(ver)
    _SUB_OPCODE_FOR_NAME[_CHAMY_NAME] = row
    op = DveOp(_CHAMY_NAME, spec, subdim=False, uops_sha=shas)
    OPS.append(op)
    CUSTOM_DVE_SPECS[_CHAMY_NAME] = spec
    return op


_MIN2_NAME = "MIN2_MINRED_ANT"


def _min2_ref(in0, in1, c0, c1, c2):
    P_ = in0.shape[0]
    body = np.minimum(in0.astype(np.float32),
                      in1.astype(np.float32)).astype(np.float32)
    b2 = body.reshape(P_, -1)
    c1 = np.asarray(c1, np.float32).reshape(-1, 1)
    acc = np.minimum(b2.min(axis=-1, keepdims=True), c1)
    return body, acc


def _min2_op():
    """Register (idempotently) the dual-stream min(a,b) + min-reduce op."""
    from concourse.dve_ops import (CUSTOM_DVE_SPECS, OPS,
                                   _SUB_OPCODE_FOR_NAME, DveOp)
    from concourse.dve_spec import C1, Spec, Src0, Src1, lower, minn
    from concourse.dve_uop import DveOpSpec

    if _MIN2_NAME in _SUB_OPCODE_FOR_NAME:
        return next(o for o in OPS if o.name == _MIN2_NAME)
    spec = Spec(body=minn(Src0, Src1), accum=minn, accum_init=C1,
                reference=_min2_ref)
    row = 1 + len(OPS)
    shas = {}
    for ver in ("v3", "v4"):
        s = DveOpSpec(name=_MIN2_NAME, opcode=row,
                      uops=lower(spec, ver=ver), rd1_en=True)
        shas[ver] = s.sha(ver)
    _SUB_OPCODE_FOR_NAME[_MIN2_NAME] = row
    op = DveOp(_MIN2_NAME, spec, subdim=False, uops_sha=shas)
    OPS.append(op)
    CUSTOM_DVE_SPECS[_MIN2_NAME] = spec
    return op


def _body(nc, tc, tile, mybir, tpd, bct, bcn, outx, outy):
    f32 = mybir.dt.float32
    bf16 = mybir.dt.bfloat16
    Alu = mybir.AluOpType
    Act = mybir.ActivationFunctionType
    X = mybir.AxisListType.X

    with tc.tile_pool(name="consts", bufs=1) as consts, \
         tc.tile_pool(name="work", bufs=4) as work, \
         tc.tile_pool(name="bcast", bufs=2) as bcast, \
         tc.tile_pool(name="dsqp", bufs=3) as dsqp:
        bct_sb = consts.tile([128, P], f32, tag="bct")
        nc.sync.dma_start(bct_sb[:], bct)
        bcn_sb = consts.tile([128, 2 * N], f32, tag="bcn")
        nc.sync.dma_start(bcn_sb[:], bcn)
        # load points + build mask in column groups so the cham_y stream
        # can start as soon as the first group's t_adj is ready
        tp_sb = consts.tile([128, COLS], f32, tag="tp")
        valid = consts.tile([128, COLS], f32, tag="valid")
        tmp = consts.tile([128, COLS], f32, tag="tmp")
        t_adj = consts.tile([128, COLS], f32, tag="tadj")
        tpd_pc = tpd.rearrange("(p c) -> p c", p=128)
        G = COLS // 4
        for g in range(4):
            sl = slice(g * G, (g + 1) * G)
            nc.sync.dma_start(tp_sb[:, sl], tpd_pc[:, sl])
            # valid = (t >= 0.001) as 1.0/0.0
            nc.vector.tensor_scalar(valid[:, sl], tp_sb[:, sl], 0.001, None,
                                    op0=Alu.is_ge)
            # t_adj = t + (1-valid)*1e9
            nc.vector.tensor_scalar(tmp[:, sl], valid[:, sl], -1e9, 1e9,
                                    op0=Alu.mult, op1=Alu.add)
            nc.vector.tensor_add(t_adj[:, sl], tmp[:, sl], tp_sb[:, sl])

        # ---- cham_x: [part=bins] big-FD pipeline ----
        # masked t as flat rows in DRAM (reshape [128,300] -> [4,9600]
        # crosses the partition/free boundary, so bounce via DRAM), then
        # DMA-broadcast each batch row to all 128 partitions
        tscratch = nc.dram_tensor("tscratch", [N * L_LOC], f32,
                                  kind="Internal").ap()
        nc.sync.dma_start(tscratch.rearrange("(p c) -> p c", p=128), t_adj[:])
        chx = consts.tile([128, 2 * N], f32, tag="chx")
        min2_op = _min2_op()
        chamy_op = _chamy_op()

        H = L_LOC // 2  # 4800
        for n in range(N):
            tbc = bcast.tile([128, L_LOC], f32, tag="tbc")
            nc.sync.dma_start(
                tbc[:], tscratch[n * L_LOC:(n + 1) * L_LOC]
                .partition_broadcast(128))
            for c in range(2):
                # (t - bc_p)^2 ; bias = -bc chunk column
                dsq = dsqp.tile([128, L_LOC], bf16, tag="dsq")
                nc.scalar.activation(dsq[:], tbc[:], Act.Square,
                                     bias=bcn_sb[:, n * 2 + c:n * 2 + c + 1],
                                     scale=1.0)
                # min-reduce: one dual-stream min(a,b)+min-accum custom op
                tr1 = dsqp.tile([128, H], bf16, tag="tr1")
                nc.vector._custom_dve(min2_op, out=tr1[:], in0=dsq[:, 0:H],
                                      in1=dsq[:, H:L_LOC], s1=3.0e38,
                                      accum_out=chx[:, n * 2 + c:n * 2 + c + 1])

        # ---- cham_y: per-point min over 256 bins of (bc - t)^2 ----
        # fused dual-stream custom DVE op, one per point column
        dy = consts.tile([128, COLS], f32, tag="dy")
        for j in range(COLS):
            scr = work.tile([128, P // 2], f32, tag="scr")
            nc.vector._custom_dve(chamy_op, out=scr[:],
                                  in0=bct_sb[:, 0:P // 2],
                                  in1=bct_sb[:, P // 2:P],
                                  s0=tp_sb[:, j:j + 1], s1=3.0e38,
                                  accum_out=dy[:, j:j + 1])

        # dy * valid, summed; plus valid count
        dym = consts.tile([128, COLS], f32, tag="dym")
        nc.vector.tensor_mul(dym[:], dy[:], valid[:])
        osum = consts.tile([128, 2], f32, tag="osum")
        nc.vector.tensor_reduce(osum[:, 0:1], dym[:], axis=X, op=Alu.add)
        nc.vector.tensor_reduce(osum[:, 1:2], valid[:], axis=X, op=Alu.add)

        # outputs on the SWDGE path so they never block the sync queue
        nc.gpsimd.dma_start(outx, chx[:])
        nc.gpsimd.dma_start(outy, osum[:])


def _build_program():
    import concourse.bacc as bacc
    import concourse.tile as tile
    from concourse import mybir

    f32 = mybir.dt.float32

    nc = bacc.Bacc("TRN2", target_bir_lowering=False, debug=False,
                   num_devices=N_CORES)
    tpd = nc.dram_tensor("tpd", [N * L_LOC], f32, kind="ExternalInput").ap()
    bct = nc.dram_tensor("bct", [128, P], f32, kind="ExternalInput").ap()
    bcn = nc.dram_tensor("bcn", [128, 2 * N], f32, kind="ExternalInput").ap()
    outx = nc.dram_tensor("outx", [128, 2 * N], f32,
                          kind="ExternalOutput").ap()
    outy = nc.dram_tensor("outy", [128, 2], f32, kind="ExternalOutput").ap()

    with tile.TileContext(nc) as tc:
        _body(nc, tc, tile, mybir, tpd, bct, bcn, outx, outy)
    nc.compile()
    return nc


def _get_program():
    if "nc" not in _CACHE:
        _CACHE["nc"] = _build_program()
    return _CACHE["nc"]


def make_inputs(bins, target_depth_maps):
    bins = np.asarray(bins, dtype=np.float32)
    tdm = np.asarray(target_depth_maps, dtype=np.float32)
    bc = 0.5 * (bins[:, 1:] + bins[:, :-1])  # [4, 256]
    bct = np.ascontiguousarray(bc[np.arange(128) // PARTS_PER_BATCH])
    # bcn[p, n*2+c] = -bc[n, c*128+p]
    bcn = np.empty((128, 2 * N), dtype=np.float32)
    for n in range(N):
        for c in range(2):
            bcn[:, n * 2 + c] = -bc[n, c * 128:(c + 1) * 128]
    tp = tdm.reshape(N, L)
    in_maps = []
    for c in range(N_CORES):
        shard = np.ascontiguousarray(
            tp[:, c * L_LOC:(c + 1) * L_LOC]).reshape(-1)
        in_maps.append({"tpd": shard, "bct": bct, "bcn": bcn})
    return in_maps


def combine(outs):
    accx = np.stack([o["outx"] for o in outs])  # [8, 128, 2N]
    osum = np.stack([o["outy"] for o in outs])  # [8, 128, 2]
    total = np.float64(0.0)
    for n in range(N):
        # cham_x: min over cores of per-bin d^2 mins, both chunks
        mins = accx[:, :, n * 2:n * 2 + 2].min(axis=0)  # [128, 2]
        cham_x = mins.mean()
        sl = slice(n * PARTS_PER_BATCH, (n + 1) * PARTS_PER_BATCH)
        dsum = osum[:, sl, 0].sum()
        cnt = osum[:, sl, 1].sum()
        cham_y = dsum / cnt
        total += cham_x + cham_y
    return np.array(total / N, dtype=np.float32)


def kernel(bins, target_depth_maps):
    from concourse.bass_utils import run_bass_kernel_spmd

    in_maps = make_inputs(bins, target_depth_maps)
    nc = _get_program()
    res = run_bass_kernel_spmd(nc, in_maps, core_ids=list(range(N_CORES)))
    return combine(res.results)

